# revision 7
# baseline (speedup 1.0000x reference)
"""Trainium2 Bass kernel for nn_AttentionTSSA — v3.

Math per (batch b, head h, channel c, position n), N = T*V = 1600:
  w   = Wqkv @ x_b                     # [C, N]
  s   = w^2 ; D = cumsum_n(s) + eA ; u = s / D
  R_h = sum_{c in h} u                 # PE indicator matmul
  E   = exp(temp_h * R + DH*db_h*temp_h) ; S = sum_h E ; Pi = E / S
  CP  = cumsum_n(Pi) + 1e-8 ; alpha = Pi * CP
  Z   = cumsum_n(s*Pi + Pi) + eZ       # = cumsum(q) + CP
  y   = w * alpha / Z                  # minus folded into -Wproj
  out = Wproj @ y + bproj

v3 structure: four fused custom-DVE ops do the heavy lifting
  AOP : u = s * recip1NR(cumsum(s)+eA)        (1 pass, 8 stages)
  ZOP : Z = cumsum(s*Pi + Pi) + eZ            (1 pass, Pi read from PSUM,
        two 800-col halves chained via scan-init carry scalar)
  MDIV: out = a * recip1NR(b)                 (Pi = E/S in the softmax)
  CPAL: alpha = Pi * (cumsum(Pi)+1e-8)        (2 stages)
Engines: PE matmuls f16 (Wqkv/Wproj/R/bcPi/sumexp); Act evacuates PSUM
(wb, ot(+bias), exp, S) and computes 1/Z via a direct Reciprocal
activation (bass wrapper bypassed; ~3e-4 rel err, fine at 2e-2 gate);
Pool does t=wb*alB and issues the x-load DMAs (SWDGE); DVE runs the
custom ops plus y = t*(1/Z).  x is cast to f16 on the host (halves HBM
read traffic).  alpha is broadcast head->channel by a single 4-row
interleaved partition_broadcast DMA per tile (see channel permutation
below); Pi is broadcast by an indicator matmul consumed from PSUM.
Schedule: A(g+1) batches and the RS softmax phase are interleaved with
B(g) so PE/Act/DVE stay fed across the group barriers; Pi = E/S reads
S straight from PSUM (no evac); group-boundary t-mults run on the
then-idle DVE instead of Pool.  TimelineSim 117,836 ns/core vs 257,327
ns baseline (2.18x); DVE ~84 us busy is the floor.

Channel permutation: partition p of j-tile holds logical channel
pi_j(p) = j*128 + (p%4)*32 + p//4  (head = 4j + p%4), so the alpha
broadcast is a single 4-row interleaved partition_broadcast DMA per
tile. Wqkv cols / Wproj rows / indicators are permuted host-side.

Sharding: data parallel over B: 64 batches -> 8 cores x 8 batches.
"""

import numpy as np

B, C, T, V = 64, 256, 64, 25
H = 8
DH = C // H                # 32
N = T * V                  # 1600
NCORES = 8
BPC = B // NCORES          # 8 batches per core
NB = C // 128              # 2 channel blocks
HPB = H // NB              # 4 heads per block
GRP = 4                    # batches per softmax group
NGRP = BPC // GRP          # 2
EPS_A = 1e-12              # D-scan init (fp32 internal)
EPS_Z = 6e-5               # Z-scan init (f16-safe output)
RC0 = -0.23549792          # recip seed Chebyshev consts
RC1 = 2.0017324
SL4 = [(0, 512), (512, 512), (1024, 512), (1536, 64)]   # 1600 psum slices
SL2 = [(0, 512), (512, 288)]                            # 800 psum slices

_CACHE = {}


def _register_custom_ops():
    import concourse.dve_ops as dve_ops
    from concourse.dve_spec import (
        Spec, Src0, Src1, C0, C1, C2, Bin, AluOp, lower, sq, scan,
        _has_src1,
    )
    from concourse.dve_uop import DveOpSpec

    have = {op.name for op in dve_ops.OPS}
    if "TSSA_AOP" in have:
        return [next(op for op in dve_ops.OPS if op.name == n)
                for n in ("TSSA_AOP", "TSSA_ZOP", "TSSA_MDIV", "TSSA_CPAL")]

    def _ref_aop(in0, in1, c0, c1, c2):
        s = np.float32(in0) ** 2
        return s / (np.cumsum(s, axis=-1) + c2)

    def _ref_zop(in0, in1, c0, c1, c2):
        q = np.float32(in0) ** 2 * in1 + in1
        return np.cumsum(q, axis=-1) + c0

    def _ref_mdiv(in0, in1, c0, c1, c2):
        return np.float32(in0) / np.float32(in1)

    _s = sq(Src0)
    _D = scan(AluOp.ADD, _s, init=C2)
    _nD = Bin(AluOp.BITWISE_NOT, _D, _D)
    _y0 = _nD * C0
    _y1 = _y0 * (C1 - _D * _y0)
    AOP_SPEC = Spec(body=_s * _y1, reference=_ref_aop)

    _q = sq(Src0) * Src1 + Src1
    ZOP_SPEC = Spec(body=scan(AluOp.ADD, _q, init=C0), reference=_ref_zop)

    _nB = Bin(AluOp.BITWISE_NOT, Src1, Src1)
    _b0 = _nB * C0
    _b1 = _b0 * (C1 - Src1 * _b0)
    MDIV_SPEC = Spec(body=Src0 * _b1, reference=_ref_mdiv)

    def _ref_cpal(in0, in1, c0, c1, c2):
        cp = np.cumsum(np.float32(in0), axis=-1) + c0
        return np.float32(in0) * cp

    CPAL_SPEC = Spec(body=Src0 * scan(AluOp.ADD, Src0, init=C0),
                     reference=_ref_cpal)

    ops = []
    for name, spec in (("TSSA_AOP", AOP_SPEC), ("TSSA_ZOP", ZOP_SPEC),
                       ("TSSA_MDIV", MDIV_SPEC), ("TSSA_CPAL", CPAL_SPEC)):
        row = dve_ops._CUSTOM_DVE_ROW_BASE + len(dve_ops.OPS)
        shas = {}
        for ver in ("v3", "v4"):
            uops = lower(spec, ver=ver)
            shas[ver] = DveOpSpec(
                name=name, opcode=row, uops=uops, rd1_en=_has_src1(spec)
            ).sha(ver)
        op = dve_ops.DveOp(name, spec, False, shas)
        dve_ops.OPS.append(op)
        dve_ops._SUB_OPCODE_FOR_NAME[name] = row
        dve_ops.CUSTOM_DVE_SPECS[name] = spec
        ops.append(op)
    return ops


def _build():
    import concourse.bass as bass
    import concourse.tile as tile
    from concourse import bacc, mybir

    dt = mybir.dt
    AF = mybir.ActivationFunctionType
    OP = mybir.AluOpType
    F16 = dt.float16
    F32 = dt.float32
    R = dt.float32r
    AOP, ZOP, MDIV, CPAL = _register_custom_ops()

    nc = bacc.Bacc("TRN2", target_bir_lowering=False, debug=False)

    x_d = nc.dram_tensor("x", [BPC, C, N], F16, kind="ExternalInput").ap()
    cpack_d = nc.dram_tensor("cpack", [128, 2 * NB * C + DH * NB + NB * 128],
                             F16, kind="ExternalInput").ap()
    spack_d = nc.dram_tensor("spack", [128, 2 + NB], F32,
                             kind="ExternalInput").ap()
    sumexp_d = nc.dram_tensor("sumexp", [128, 128], R,
                              kind="ExternalInput").ap()
    out_d = nc.dram_tensor("out", [BPC, C, N], F16, kind="ExternalOutput").ap()
    alscr_d = nc.dram_tensor("alscr", [NGRP, 128, N], F16, kind="Internal").ap()

    with tile.TileContext(nc) as tc:
        with (
            tc.tile_pool(name="const", bufs=1) as pc,
            tc.tile_pool(name="xin", bufs=3) as px,
            tc.tile_pool(name="wst", bufs=14) as pw,            # wb16 ring
            tc.tile_pool(name="ust", bufs=2 * GRP + 2) as pu,   # u16 ring
            tc.tile_pool(name="bph", bufs=3) as pb,             # Z/t tiles
            tc.tile_pool(name="alb", bufs=7) as pa,            # alB tiles
            tc.tile_pool(name="ytile", bufs=6) as py,           # yt
            tc.tile_pool(name="otile", bufs=3) as po,           # out16
            tc.tile_pool(name="hstk", bufs=2) as ph,            # Pi16
            tc.tile_pool(name="rsw", bufs=1) as pr,             # RS scratch
            tc.tile_pool(name="psA", bufs=2, space="PSUM") as pQ,   # Wqkv/Wproj
            tc.tile_pool(name="psB", bufs=2, space="PSUM") as pB2,  # PiB/R/S
        ):
            NC16 = 2 * NB * C + DH * NB + NB * 128
            cpack = pc.tile([128, NC16], F16, tag="cpack")
            nc.sync.dma_start(cpack[:, 0:NB * C], cpack_d[:, 0:NB * C])
            nc.sync.dma_start(cpack[:, NB * C:], cpack_d[:, NB * C:])
            spack = pc.tile([128, 2 + NB], F32, tag="spack")
            nc.sync.dma_start(spack, spack_d)
            sumexp = pc.tile([128, 128], R, tag="sumexp")
            nc.sync.dma_start(sumexp, sumexp_d)
            wqkvT = cpack[:, 0:NB * C].rearrange("p (k c) -> p k c", k=NB)
            wprojTn = cpack[:, NB * C:2 * NB * C].rearrange(
                "p (k c) -> p k c", k=NB)
            o = 2 * NB * C
            rlhs = cpack[:, o:o + DH * NB].rearrange("p (j m) -> p j m", j=NB)
            o += DH * NB
            bc = cpack[:, o:o + NB * 128].rearrange("p (j c) -> p j c", j=NB)
            ebias = spack[:, 0:1]
            escale = spack[:, 1:2]
            bproj = spack[:, 2:2 + NB]

            wtiles = {}   # (bi, j) -> wb16 [128, N]
            utiles = {}   # (bi, j) -> u16 [128, N]

            def act_recip(out, in_):
                ins = [nc.scalar.lower_ap(in_)]
                for arg in (0.0, 1.0, 0.0):
                    ins.append(mybir.ImmediateValue(dtype=F32, value=arg))
                nc.scalar.add_instruction(
                    mybir.InstActivation(
                        name=nc.get_next_instruction_name(),
                        func=AF.Reciprocal, ins=ins,
                        outs=[nc.scalar.lower_ap(out)]))

            xqueue = {}

            def fetch_x(bi):
                if bi >= BPC or bi in xqueue:
                    return
                xt = px.tile([128, NB, N], F16, tag="xt")
                for k in range(NB):
                    nc.gpsimd.dma_start(
                        xt[:, k, :], x_d[bi, k * 128:(k + 1) * 128, :])
                xqueue[bi] = xt

            def emit_A(g, bl):
                bi = g * GRP + bl
                fetch_x(bi)
                xt = xqueue.pop(bi)
                fetch_x(bi + 2)
                for j in range(NB):
                    wb = pw.tile([128, N], F16, tag="wb")
                    wtiles[(bi, j)] = wb
                    Wps = [pQ.tile([128, 800], F32, tag="psA",
                                   name=f"Wp{hi}") for hi in range(2)]
                    for k in range(NB):
                        for hi, hoff in enumerate((0, 800)):
                            for soff, slen in SL2:
                                nc.tensor.matmul(
                                    Wps[hi][:, soff:soff + slen],
                                    wqkvT[:, k, j * 128:(j + 1) * 128],
                                    xt[:, k, hoff + soff:hoff + soff + slen],
                                    start=(k == 0), stop=(k == NB - 1),
                                )
                    for hi, hoff in enumerate((0, 800)):
                        nc.scalar.copy(wb[:, hoff:hoff + 800], Wps[hi])
                    u16 = pu.tile([128, N], F16, tag="u16")
                    utiles[(bi, j)] = u16
                    nc.vector._custom_dve(AOP, out=u16, in0=wb,
                                          s0=RC0, s1=RC1, imm2=EPS_A)

            def emit_RS_a(g):
                E32 = pr.tile([128, N], R, tag="E32")
                for hoff, hlen in ((0, 800), (800, 800)):
                    Rp = pB2.tile([128, 800], F32, tag="psB")
                    for j in range(NB):
                        for bl in range(GRP):
                            bi = g * GRP + bl
                            for soff, slen in SL2:
                                nc.tensor.matmul(
                                    Rp[DH * bl:DH * (bl + 1),
                                       soff:soff + slen],
                                    rlhs[:, j, :],
                                    utiles[(bi, j)][:,
                                        hoff + soff:hoff + soff + slen],
                                    start=(j == 0), stop=(j == NB - 1),
                                    tile_position=(0, DH * bl),
                                )
                    nc.scalar.activation(
                        E32[:, hoff:hoff + hlen], Rp,
                        AF.Exp, bias=ebias[:, 0:1], scale=escale[:, 0:1])
                for bl in range(GRP):
                    for j in range(NB):
                        del utiles[(g * GRP + bl, j)]
                return E32

            def emit_RS_b(g, E32):
                Pi16 = ph.tile([128, N], F16, tag="Pi16")
                al16 = pr.tile([128, N], F16, tag="al16")
                for hoff, hlen in ((0, 800), (800, 800)):
                    Sp = pB2.tile([128, 800], F32, tag="psB")
                    for soff, slen in SL2:
                        nc.tensor.matmul(Sp[:, soff:soff + slen], sumexp,
                                         E32[:, hoff + soff:hoff + soff + slen],
                                         start=True, stop=True)
                    nc.vector._custom_dve(
                        MDIV, out=Pi16[:, hoff:hoff + hlen],
                        in0=E32.bitcast(F32)[:, hoff:hoff + hlen],
                        in1=Sp, s0=RC0, s1=RC1)
                nc.vector._custom_dve(CPAL, out=al16, in0=Pi16, s0=1e-8)
                nc.sync.dma_start(alscr_d[g], al16)
                albs = {}
                for j in range(NB):
                    r0 = HPB * j
                    alB = pa.tile([128, N], F16, tag="alB")
                    nc.sync.dma_start(
                        alB,
                        alscr_d[g, r0:r0 + HPB, :].partition_broadcast(DH))
                    albs[(0, j)] = alB
                return (Pi16, albs)

            def emit_B_a(g, bl, Pi16, albs):
                bi = g * GRP + bl
                if bl + 1 < GRP:
                    for j in range(NB):
                        r0 = DH * (bl + 1) + HPB * j
                        alB = pa.tile([128, N], F16, tag="alB")
                        nc.sync.dma_start(
                            alB,
                            alscr_d[g, r0:r0 + HPB, :]
                            .partition_broadcast(DH))
                        albs[(bl + 1, j)] = alB
                yts = []
                Zs, Ts = [], []
                for j in range(NB):
                    wb = wtiles[(bi, j)]
                    Z32 = pb.tile([128, N], F32, tag="Z32")
                    Zs.append(Z32)
                    for hoff in (0, 800):
                        PiBp = pB2.tile([128, 800], F32, tag="psB")
                        for soff, slen in SL2:
                            nc.tensor.matmul(
                                PiBp[:, soff:soff + slen],
                                bc[DH * bl:DH * bl + H, j, :],
                                Pi16[DH * bl:DH * bl + H,
                                     hoff + soff:hoff + soff + slen],
                                start=True, stop=True,
                                tile_position=(DH * bl, 0))
                        nc.vector._custom_dve(
                            ZOP, out=Z32[:, hoff:hoff + 800],
                            in0=wb[:, hoff:hoff + 800], in1=PiBp,
                            s0=(EPS_Z if hoff == 0
                                else Z32[:, hoff - 1:hoff]))
                for j in range(NB):
                    alB = albs.pop((bl, j))
                    wb = wtiles.pop((bi, j))
                    t16 = pb.tile([128, N], F16, tag="t16")
                    Ts.append(t16)
                    if bl == 0:
                        # group-boundary: DVE is idle waiting on alB anyway;
                        # its 894ns mult beats Pool's 3270ns on the chain
                        nc.vector.tensor_tensor(t16, wb, alB, OP.mult)
                    else:
                        nc.gpsimd.tensor_tensor(t16, wb, alB, OP.mult)
                for j in range(NB):
                    rZ = pb.tile([128, N], F16, tag="rZ")
                    act_recip(rZ, Zs[j])
                    yt = py.tile([128, N], F16, tag="yt")
                    yts.append(yt)
                    nc.vector.tensor_tensor(yt, Ts[j], rZ, OP.mult)
                return yts

            def emit_B_b(g, bl, yts):
                bi = g * GRP + bl
                for jo in range(NB):
                    ot = po.tile([128, N], F16, tag="ot")
                    Ops = [pQ.tile([128, 800], F32, tag="psA",
                                   name=f"Op{hi}") for hi in range(2)]
                    for k in range(NB):
                        for hi, hoff in enumerate((0, 800)):
                            for soff, slen in SL2:
                                nc.tensor.matmul(
                                    Ops[hi][:, soff:soff + slen],
                                    wprojTn[:, k, jo * 128:(jo + 1) * 128],
                                    yts[k][:, hoff + soff:hoff + soff + slen],
                                    start=(k == 0), stop=(k == NB - 1),
                                )
                    for hi, hoff in enumerate((0, 800)):
                        nc.scalar.activation(ot[:, hoff:hoff + 800],
                                             Ops[hi], AF.Identity,
                                             bias=bproj[:, jo:jo + 1],
                                             scale=1.0)
                    for hoff in (0, 800):
                        nc.sync.dma_start(
                            out_d[bi, jo * 128:(jo + 1) * 128,
                                  hoff:hoff + 800],
                            ot[:, hoff:hoff + 800])

            assert NGRP == 2

            def emit_B(g, bl, Pi16, albs):
                emit_B_b(g, bl, emit_B_a(g, bl, Pi16, albs))

            for bl in range(GRP):
                emit_A(0, bl)
            emit_A(1, 0)
            stk0 = emit_RS_b(0, emit_RS_a(0))
            emit_A(1, 1)
            emit_B(0, 0, *stk0)
            emit_A(1, 2)
            emit_B(0, 1, *stk0)
            emit_A(1, 3)
            emit_B(0, 2, *stk0)
            stk1 = emit_RS_b(1, emit_RS_a(1))
            emit_B(0, 3, *stk0)
            for bl in range(GRP):
                emit_B(1, bl, *stk1)

    nc.compile()
    return nc


def _perm(j):
    """partition p of j-tile -> logical channel index."""
    p = np.arange(128)
    return j * 128 + (p % HPB) * DH + p // HPB


def _host_constants(Wqkv, temp, denom_bias, Wproj, bproj):
    f32 = np.float32
    f16 = np.float16
    WqT = Wqkv.T.astype(f32)              # [in, out]
    WpTn = (-Wproj.T).astype(f32)         # [in, out]
    wqkvT = np.zeros((NB, 128, C), f32)   # [k, p_in, out-col within j blocks]
    wprojTn = np.zeros((NB, 128, C), f32)
    perms = [_perm(j) for j in range(NB)]
    for k in range(NB):
        for j in range(NB):
            # Wqkv: out channel permuted within each j block of 128 cols
            wqkvT[k, :, j * 128:(j + 1) * 128] = \
                WqT[k * 128:(k + 1) * 128, perms[j]]
        # Wproj: IN channel (partition of yts[k]) permuted; out cols plain
        wprojTn[k] = WpTn[perms[k], :]
    temp = temp.reshape(H).astype(f32)
    denom_bias = denom_bias.reshape(H).astype(f32)
    rlhs = np.zeros((NB, 128, DH), f16)
    bc = np.zeros((128, NB, 128), f16)
    for j in range(NB):
        for p in range(128):
            h = HPB * j + p % HPB          # global head 0..7
            rlhs[j, p, h] = 1.0            # row h within the 32-row block
            for q in range(GRP):
                bc[DH * q + h, j, p] = 1.0
    sumexp = np.zeros((128, 128), f32)
    for q in range(GRP):
        r0 = DH * q
        sumexp[r0:r0 + H, r0:r0 + H] = 1.0
    ebias = np.zeros((128, 1), f32)
    escale = np.zeros((128, 1), f32)
    for q in range(GRP):
        for h in range(H):
            ebias[DH * q + h, 0] = DH * denom_bias[h] * temp[h]
            escale[DH * q + h, 0] = temp[h]
    bproj2 = np.ascontiguousarray(bproj.reshape(NB, 128).T).astype(f32)

    # pack: cpack [128, NB*C*2 + NB*DH + NB*128] f16 (tile layout p,(k c))
    cpack = np.concatenate([
        wqkvT.astype(f16).transpose(1, 0, 2).reshape(128, NB * C),
        wprojTn.astype(f16).transpose(1, 0, 2).reshape(128, NB * C),
        rlhs.transpose(1, 0, 2).reshape(128, NB * DH),
        bc.transpose(0, 1, 2).reshape(128, NB * 128),
    ], axis=1)
    spack = np.concatenate([ebias, escale, bproj2], axis=1).astype(np.float32)
    return {"cpack": np.ascontiguousarray(cpack),
            "spack": np.ascontiguousarray(spack),
            "sumexp": sumexp}


def kernel(x, Wqkv, temp, denom_bias, Wproj, bproj, *, _run=None):
    x = np.asarray(x, np.float32)
    Wqkv = np.asarray(Wqkv, np.float32)
    temp = np.asarray(temp, np.float32)
    denom_bias = np.asarray(denom_bias, np.float32)
    Wproj = np.asarray(Wproj, np.float32)
    bproj = np.asarray(bproj, np.float32)

    if "nc" not in _CACHE:
        _CACHE["nc"] = _build()
    nc = _CACHE["nc"]

    consts = _host_constants(Wqkv, temp, denom_bias, Wproj, bproj)
    xr = x.reshape(B, C, N)
    in_maps = []
    for core in range(NCORES):
        m = dict(consts)
        m["x"] = np.ascontiguousarray(
            xr[core * BPC:(core + 1) * BPC]).astype(np.float16)
        in_maps.append(m)

    if _run is None:
        from concourse import bass_utils
        res = bass_utils.run_bass_kernel_spmd(nc, in_maps, list(range(NCORES)))
        outs = [r["out"] for r in res.results]
    else:
        outs = _run(nc, in_maps)

    # un-permute channels: out partition p of jo-block is plain channel
    # (no permutation on outputs), so direct reshape.
    out = np.concatenate(outs, axis=0).reshape(B, C, T, V)
    return out.astype(np.float32)
